# revision 12
# baseline (speedup 1.0000x reference)
import sys

sys.path.insert(0, "/opt/trn_rl_repo")

import numpy as np
import concourse.bass as bass
import concourse.mybir as mybir
from concourse import tile, bacc, bass_isa
from concourse.bass_utils import run_bass_kernel_spmd

# Problem dims (hardcoded per contract)
B, L, H, E, VOCAB = 64, 2048, 1024, 1024, 32000
NC = 8
BS = B // NC      # 8  batch rows per core (attention)
HS = H // NC      # 128 hidden rows per core (GRU/Wc shard)
VS = VOCAB // NC  # 4000 vocab rows per core (Wout shard)
KH = H // 128     # 8  k-chunks over H/E
KC = 2 * H // 128 # 16 k-chunks over 2H (Wc contraction)
NT = L // 128     # 16 L-tiles of 128
TCH = 4           # L-tiles per enc chunk
NCH = NT // TCH   # 4 chunks per batch row
NV = 8            # vocab n-tiles per core
VT = VS // NV     # 500
F32 = mybir.dt.float32
AX = mybir.AxisListType
OP = mybir.AluOpType
ACT = mybir.ActivationFunctionType

_built = None


def _build():
    nc = bacc.Bacc("TRN2", target_bir_lowering=False, debug=False, num_devices=NC)

    # ---- dram I/O ----
    d_xT = nc.dram_tensor("xT", [E, B], F32, kind="ExternalInput")
    d_hp0T = nc.dram_tensor("hp0T", [H, B], F32, kind="ExternalInput")
    d_hp1T = nc.dram_tensor("hp1T", [H, B], F32, kind="ExternalInput")
    d_hp0c = nc.dram_tensor("hp0c", [HS, B], F32, kind="ExternalInput")
    d_hp1c = nc.dram_tensor("hp1c", [HS, B], F32, kind="ExternalInput")
    d_wih0 = nc.dram_tensor("wih0T", [E, 3 * HS], F32, kind="ExternalInput")
    d_whh0 = nc.dram_tensor("whh0T", [H, 3 * HS], F32, kind="ExternalInput")
    d_wih1 = nc.dram_tensor("wih1T", [H, 3 * HS], F32, kind="ExternalInput")
    d_whh1 = nc.dram_tensor("whh1T", [H, 3 * HS], F32, kind="ExternalInput")
    d_bias0 = nc.dram_tensor("bias0", [HS, 4], F32, kind="ExternalInput")
    d_bias1 = nc.dram_tensor("bias1", [HS, 4], F32, kind="ExternalInput")
    d_wcT = nc.dram_tensor("wcT", [2 * H, HS], F32, kind="ExternalInput")
    d_bcb = nc.dram_tensor("bcb", [HS, 1], F32, kind="ExternalInput")
    d_woutT = nc.dram_tensor("woutT", [H, VS], F32, kind="ExternalInput")
    d_boutb = nc.dram_tensor("boutb", [1, VS], F32, kind="ExternalInput")
    d_enc = nc.dram_tensor("enc", [BS, L, H], F32, kind="ExternalInput")
    d_ident = nc.dram_tensor("ident", [128, 128], F32, kind="ExternalInput")
    d_sel = nc.dram_tensor("sel", [B, BS], F32, kind="ExternalInput")

    d_out = nc.dram_tensor("out_c", [B, VS], F32, kind="ExternalOutput")
    d_h0o = nc.dram_tensor("h0o", [B, HS], F32, kind="ExternalOutput")
    d_h1o = nc.dram_tensor("h1o", [B, HS], F32, kind="ExternalOutput")
    d_attnw = nc.dram_tensor("attnw", [BS, L], F32, kind="ExternalOutput")

    with tile.TileContext(nc) as tc:
        with (
            tc.tile_pool(name="const", bufs=1) as cp,
            tc.tile_pool(name="gw", bufs=6) as gw,
            tc.tile_pool(name="enc", bufs=6) as ep,
            tc.tile_pool(name="work", bufs=2) as rp,
            tc.tile_pool(name="scratch", bufs=1) as sp,
            tc.tile_pool(name="wout", bufs=3) as wop,
            tc.tile_pool(name="dram", bufs=1, space="DRAM") as dp,
        ):
            # ---------- load constants / weights ----------
            ident = cp.tile([128, 128], F32, tag="ident")
            nc.sync.dma_start(ident[:], d_ident[:])
            sel = cp.tile([B, BS], F32, tag="sel")
            nc.sync.dma_start(sel[:], d_sel[:])
            ones1 = cp.tile([1, B], F32, tag="ones1")
            nc.vector.memset(ones1[:], 1.0)
            bias0 = cp.tile([HS, 4], F32, tag="bias0")
            nc.sync.dma_start(bias0[:], d_bias0[:])
            bias1 = cp.tile([HS, 4], F32, tag="bias1")
            nc.sync.dma_start(bias1[:], d_bias1[:])
            bcb = cp.tile([HS, 1], F32, tag="bcb")
            nc.sync.dma_start(bcb[:], d_bcb[:])

            def load_T(dram, D, tag):  # [D, B] dram -> [128, D//128, B] sbuf
                t = cp.tile([128, D // 128, B], F32, tag=tag)
                nc.sync.dma_start(t[:], dram[:].rearrange("(k p) b -> p k b", p=128))
                return t

            xT = load_T(d_xT, E, "xT")
            hp0T = load_T(d_hp0T, H, "hp0T")
            hp1T = load_T(d_hp1T, H, "hp1T")
            hp0c = cp.tile([HS, B], F32, tag="hp0c")
            nc.sync.dma_start(hp0c[:], d_hp0c[:])
            hp1c = cp.tile([HS, B], F32, tag="hp1c")
            nc.sync.dma_start(hp1c[:], d_hp1c[:])

            def allgather_T(src_sb, tag):  # [128, B] per core -> [128, NC, B] full
                bin_ = dp.tile([128, B], F32, tag=f"agi_{tag}")
                bout_ = dp.tile([NC, 128, B], F32, tag=f"ago_{tag}")
                nc.sync.dma_start(bin_[:], src_sb[:])
                nc.gpsimd.collective_compute(
                    "AllGather", OP.bypass,
                    replica_groups=[list(range(NC))],
                    ins=[bin_[:]], outs=[bout_[:]])
                full = cp.tile([128, NC, B], F32, tag=f"agf_{tag}")
                nc.sync.dma_start(full[:], bout_[:].rearrange("c p b -> p c b"))
                return full

            # ---------- GRU ----------
            def gru_layer(lid, xinT, hprevT, hprev_c, d_wih, d_whh, bias, pg):
                gi = [pg.tile([HS, B], F32, tag=f"gi{g}", name=f"gi{g}") for g in range(3)]
                gh = [pg.tile([HS, B], F32, tag=f"gh{g}", name=f"gh{g}") for g in range(3)]
                for g in range(3):
                    for dmat, out, xin in ((d_wih, gi[g], xinT), (d_whh, gh[g], hprevT)):
                        for k in range(KH):
                            wt_ = gw.tile([128, HS], F32, tag="gru_w", name="gru_w")
                            nc.sync.dma_start(
                                wt_[:],
                                dmat[k * 128:(k + 1) * 128, g * HS:(g + 1) * HS])
                            nc.tensor.matmul(
                                out[:], wt_[:], xin[:, k, :],
                                start=(k == 0), stop=(k == KH - 1))
                ghr = sp.tile([HS, B], F32, tag="ghr")
                nc.scalar.copy(ghr[:], gh[0][:])
                ghz = sp.tile([HS, B], F32, tag="ghz")
                nc.scalar.copy(ghz[:], gh[1][:])
                r = sp.tile([HS, B], F32, tag="r")
                nc.vector.tensor_add(r[:], gi[0][:], ghr[:])
                nc.scalar.activation(r[:], r[:], ACT.Sigmoid, bias=bias[:, 0:1])
                z = sp.tile([HS, B], F32, tag="z")
                nc.vector.tensor_add(z[:], gi[1][:], ghz[:])
                nc.scalar.activation(z[:], z[:], ACT.Sigmoid, bias=bias[:, 1:2])
                ghn = sp.tile([HS, B], F32, tag="ghn")
                nc.vector.tensor_scalar_add(ghn[:], gh[2][:], bias[:, 3:4])
                rhn = sp.tile([HS, B], F32, tag="rhn")
                nc.vector.tensor_mul(rhn[:], r[:], ghn[:])
                n_ = sp.tile([HS, B], F32, tag="n_")
                nc.vector.tensor_add(n_[:], gi[2][:], rhn[:])
                nc.scalar.activation(n_[:], n_[:], ACT.Tanh, bias=bias[:, 2:3])
                d = sp.tile([HS, B], F32, tag="d")
                nc.vector.tensor_sub(d[:], hprev_c[:], n_[:])
                zd = sp.tile([HS, B], F32, tag="zd")
                nc.vector.tensor_mul(zd[:], z[:], d[:])
                hnew = sp.tile([HS, B], F32, tag=f"hnew{lid}")
                nc.vector.tensor_add(hnew[:], n_[:], zd[:])
                return hnew

            with tc.tile_pool(name="psum_gru", bufs=1, space="PSUM") as pg:
                h0c = gru_layer(0, xT, hp0T, hp0c, d_wih0, d_whh0, bias0, pg)
                h0T = allgather_T(h0c, "h0")
                h1c = gru_layer(1, h0T, hp1T, hp1c, d_wih1, d_whh1, bias1, pg)
                h1T = allgather_T(h1c, "h1")

            with tc.tile_pool(name="psum_mid", bufs=1, space="PSUM") as pm:
                # hidden outputs: transpose [128, B] -> [B, 128]
                for nm, hc, dout in (("h0", h0c, d_h0o), ("h1", h1c, d_h1o)):
                    ps = pm.tile([B, 128], F32, tag=f"hout_{nm}")
                    nc.tensor.transpose(ps[:], hc[:], ident[:])
                    sb = sp.tile([B, 128], F32, tag=f"houts_{nm}")
                    nc.scalar.copy(sb[:], ps[:])
                    nc.sync.dma_start(dout[:], sb[:])
                # h1 natural layout + this core's rows
                h1nat_ps = pm.tile([B, H], F32, tag="h1nat_ps")
                for k in range(KH):
                    nc.tensor.transpose(h1nat_ps[:, k * 128:(k + 1) * 128],
                                        h1T[:, k, :], ident[:])
                h1nat = cp.tile([B, H], F32, tag="h1nat")
                nc.scalar.copy(h1nat[:, 0:512], h1nat_ps[:, 0:512])
                nc.scalar.copy(h1nat[:, 512:1024], h1nat_ps[:, 512:1024])
                hnc_ps = pm.tile([BS, H], F32, tag="hnc_ps")
                nc.tensor.matmul(hnc_ps[:, 0:512], sel[:], h1nat[:, 0:512],
                                 start=True, stop=True)
                nc.tensor.matmul(hnc_ps[:, 512:1024], sel[:], h1nat[:, 512:1024],
                                 start=True, stop=True)
                h1natc = cp.tile([BS, H], F32, tag="h1natc")
                nc.scalar.copy(h1natc[:], hnc_ps[:])

            # ---------- attention ----------
            ctx_c = sp.tile([BS, H], F32, tag="ctx_c")
            sstt = sp.tile([128, H], F32, tag="sstt")  # stt dummy out
            with tc.tile_pool(name="psum_attn", bufs=2, space="PSUM") as pa, \
                 tc.tile_pool(name="psum_attn2", bufs=2, space="PSUM") as pa2:
                for j in range(BS):
                    h1row = sp.tile([1, H], F32, tag="h1row")
                    nc.sync.dma_start(h1row[:], h1natc[j:j + 1, :])
                    h1B = rp.tile([128, H], F32, tag="h1B")
                    nc.gpsimd.partition_broadcast(h1B[:], h1row[:])
                    sbuf_s = rp.tile([128, NT], F32, tag="sbuf_s")
                    chunks = []
                    for ch in range(NCH):
                        cht = ep.tile([128, TCH, H], F32, tag="enc_ch")
                        src = d_enc[j].rearrange("(t p) h -> p t h", p=128)
                        nc.sync.dma_start(cht[:], src[:, ch * TCH:(ch + 1) * TCH, :])
                        chunks.append(cht)
                        for tl in range(TCH):
                            t = ch * TCH + tl
                            nc.vector.scalar_tensor_tensor(
                                out=sstt[:], in0=cht[:, tl, :], scalar=1.0,
                                in1=h1B[:], op0=OP.mult, op1=OP.mult,
                                accum_out=sbuf_s[:, t:t + 1])
                    # softmax over [128, NT]
                    mx = rp.tile([128, 1], F32, tag="mx")
                    nc.vector.tensor_reduce(mx[:], sbuf_s[:], axis=AX.X, op=OP.max)
                    mall = rp.tile([128, 1], F32, tag="mall")
                    nc.gpsimd.partition_all_reduce(mall[:], mx[:], channels=128,
                                                   reduce_op=bass_isa.ReduceOp.max)
                    nc.vector.tensor_scalar_sub(sbuf_s[:], sbuf_s[:], mall[:])
                    exps = rp.tile([128, NT], F32, tag="exps")
                    esum = rp.tile([128, 1], F32, tag="esum")
                    nc.scalar.activation(exps[:], sbuf_s[:], ACT.Exp, accum_out=esum[:])
                    sall = rp.tile([128, 1], F32, tag="sall")
                    nc.gpsimd.partition_all_reduce(sall[:], esum[:], channels=128,
                                                   reduce_op=bass_isa.ReduceOp.add)
                    rS = rp.tile([128, 1], F32, tag="rS")
                    nc.vector.reciprocal(rS[:], sall[:])
                    w = rp.tile([128, NT], F32, tag="w")
                    nc.vector.tensor_scalar_mul(w[:], exps[:], rS[:])
                    # attn weights out: transpose [128, NT] -> [NT, 128]
                    wt_ps = pa2.tile([NT, 128], F32, tag="wt_ps")
                    nc.tensor.transpose(wt_ps[:], w[:], ident[:])
                    wt = rp.tile([NT, 128], F32, tag="wt")
                    nc.scalar.copy(wt[:], wt_ps[:])
                    nc.sync.dma_start(
                        d_attnw[j:j + 1, :].rearrange("x (t p) -> (x t) p", p=128),
                        wt[:])
                    # context accumulate: ctx[j, :] = sum_t w[:, t].T @ enc_t
                    ctx_ps = pa.tile([1, H], F32, tag="ctx_ps")
                    for ch in range(NCH):
                        for tl in range(TCH):
                            t = ch * TCH + tl
                            for half in range(2):
                                nc.tensor.matmul(
                                    ctx_ps[0:1, half * 512:(half + 1) * 512],
                                    w[:, t:t + 1],
                                    chunks[ch][:, tl, half * 512:(half + 1) * 512],
                                    start=(t == 0), stop=(t == NT - 1),
                                    skip_group_check=True)
                    ctx_row = sp.tile([1, H], F32, tag="ctx_row")
                    nc.scalar.copy(ctx_row[:, 0:512], ctx_ps[:, 0:512])
                    nc.scalar.copy(ctx_row[:, 512:1024], ctx_ps[:, 512:1024])
                    nc.sync.dma_start(ctx_c[j:j + 1, :], ctx_row[:])

            # ---------- ctxT + allgather + Wc + Wout ----------
            with tc.tile_pool(name="psum_tail", bufs=1, space="PSUM") as ptl, \
                 tc.tile_pool(name="psum_out", bufs=2, space="PSUM") as pto:
                ctxT = sp.tile([128, KH, BS], F32, tag="ctxT")
                for hc in range(KH):
                    tp = ptl.tile([128, BS], F32, tag="ctxT_ps")
                    nc.tensor.transpose(tp[:], ctx_c[:, hc * 128:(hc + 1) * 128],
                                        ident[0:BS, 0:BS])
                    nc.scalar.copy(ctxT[:, hc, :], tp[:])
                ag3i = dp.tile([128, KH, BS], F32, tag="ag3i")
                ag3o = dp.tile([NC, 128, KH, BS], F32, tag="ag3o")
                nc.sync.dma_start(ag3i[:], ctxT[:])
                nc.gpsimd.collective_compute(
                    "AllGather", OP.bypass, replica_groups=[list(range(NC))],
                    ins=[ag3i[:]], outs=[ag3o[:]])
                ccT = cp.tile([128, KH, NC, BS], F32, tag="ccT")
                nc.sync.dma_start(ccT[:], ag3o[:].rearrange("c p h j -> p h c j"))

                # Wc (H-shard): co_cT [128, B]
                co_ps = ptl.tile([HS, B], F32, tag="co_ps")
                for k in range(KC):
                    wck = wop.tile([128, HS], F32, tag="wc_t", name="wc_t")
                    nc.sync.dma_start(wck[:], d_wcT[k * 128:(k + 1) * 128, :])
                    rhs = h1T[:, k, :] if k < KH else ccT[:, k - KH, :, :]
                    nc.tensor.matmul(co_ps[:], wck[:], rhs,
                                     start=(k == 0), stop=(k == KC - 1))
                co_c = sp.tile([HS, B], F32, tag="co_c")
                nc.scalar.activation(co_c[:], co_ps[:], ACT.Tanh, bias=bcb[:])
                coT = allgather_T(co_c, "co")

                # Wout (vocab shard)
                for n in range(NV):
                    po = pto.tile([B, VT], F32, tag="po")
                    for k in range(KH):
                        wt_ = wop.tile([128, VT], F32, tag="wout_t")
                        nc.sync.dma_start(
                            wt_[:],
                            d_woutT[k * 128:(k + 1) * 128, n * VT:(n + 1) * VT])
                        nc.tensor.matmul(po[:], coT[:, k, :], wt_[:],
                                         start=(k == 0), stop=False)
                    bt = sp.tile([1, VT], F32, tag="bout_t")
                    nc.sync.dma_start(bt[:], d_boutb[:, n * VT:(n + 1) * VT])
                    nc.tensor.matmul(po[:], ones1[:], bt[:], start=False, stop=True)
                    ob = rp.tile([B, VT], F32, tag="ob")
                    nc.scalar.copy(ob[:], po[:])
                    nc.sync.dma_start(d_out[:, n * VT:(n + 1) * VT], ob[:])

    nc.finalize()
    return nc


def _get_nc():
    global _built
    if _built is None:
        _built = _build()
    return _built


def _prep_in_maps(input_seq, last_hidden, encoder_outputs, embed_table,
                  W_ih0, W_hh0, b_ih0, b_hh0, W_ih1, W_hh1, b_ih1, b_hh1,
                  Wc, bc, Wout, bout):
    f = np.float32
    idx = np.asarray(input_seq).reshape(-1).astype(np.int64)
    x = np.asarray(embed_table)[idx].astype(f)              # [B, E]
    xT = np.ascontiguousarray(x.T)                          # [E, B]
    lh = np.asarray(last_hidden, dtype=f)
    hp0T = np.ascontiguousarray(lh[0].T)                    # [H, B]
    hp1T = np.ascontiguousarray(lh[1].T)
    enc = np.asarray(encoder_outputs, dtype=f)
    Wc_, Wout_ = np.asarray(Wc, f), np.asarray(Wout, f)
    bc_, bout_ = np.asarray(bc, f), np.asarray(bout, f)
    Wih0, Whh0 = np.asarray(W_ih0, f), np.asarray(W_hh0, f)
    Wih1, Whh1 = np.asarray(W_ih1, f), np.asarray(W_hh1, f)
    bih0, bhh0 = np.asarray(b_ih0, f), np.asarray(b_hh0, f)
    bih1, bhh1 = np.asarray(b_ih1, f), np.asarray(b_hh1, f)
    ident = np.eye(128, dtype=f)

    def gate_slice_T(W, c):
        sl = np.concatenate(
            [W[g * H + c * HS: g * H + c * HS + HS] for g in range(3)], 0)
        return np.ascontiguousarray(sl.T)

    def bias4(bi, bh, c):
        s = c * HS
        out = np.empty((HS, 4), f)
        out[:, 0] = bi[0 * H + s:0 * H + s + HS] + bh[0 * H + s:0 * H + s + HS]
        out[:, 1] = bi[1 * H + s:1 * H + s + HS] + bh[1 * H + s:1 * H + s + HS]
        out[:, 2] = bi[2 * H + s:2 * H + s + HS]
        out[:, 3] = bh[2 * H + s:2 * H + s + HS]
        return out

    in_maps = []
    for c in range(NC):
        sel = np.zeros((B, BS), f)
        sel[c * BS:(c + 1) * BS, :] = np.eye(BS, dtype=f)
        m = {
            "xT": xT, "hp0T": hp0T, "hp1T": hp1T,
            "hp0c": np.ascontiguousarray(hp0T[c * HS:(c + 1) * HS]),
            "hp1c": np.ascontiguousarray(hp1T[c * HS:(c + 1) * HS]),
            "wih0T": gate_slice_T(Wih0, c), "whh0T": gate_slice_T(Whh0, c),
            "wih1T": gate_slice_T(Wih1, c), "whh1T": gate_slice_T(Whh1, c),
            "bias0": bias4(bih0, bhh0, c), "bias1": bias4(bih1, bhh1, c),
            "wcT": np.ascontiguousarray(Wc_[c * HS:(c + 1) * HS].T),
            "bcb": np.ascontiguousarray(bc_[c * HS:(c + 1) * HS].reshape(HS, 1)),
            "woutT": np.ascontiguousarray(Wout_[c * VS:(c + 1) * VS].T),
            "boutb": np.ascontiguousarray(bout_[c * VS:(c + 1) * VS].reshape(1, VS)),
            "enc": np.ascontiguousarray(enc[c * BS:(c + 1) * BS]),
            "ident": ident, "sel": sel,
        }
        in_maps.append(m)
    return in_maps


def _assemble(results):
    f = np.float32
    output = np.concatenate([results[c]["out_c"] for c in range(NC)], axis=1).astype(f)
    hidden = np.empty((2, B, H), f)
    for c in range(NC):
        hidden[0][:, c * HS:(c + 1) * HS] = results[c]["h0o"]
        hidden[1][:, c * HS:(c + 1) * HS] = results[c]["h1o"]
    attn = np.concatenate([results[c]["attnw"] for c in range(NC)], axis=0)
    attn_weights = attn.reshape(B, 1, L).astype(f)
    return output, hidden, attn_weights


def kernel(**inputs):
    nc = _get_nc()
    in_maps = _prep_in_maps(**inputs)
    res = run_bass_kernel_spmd(nc, in_maps, list(range(NC)))
    return _assemble(res.results)
